# revision 1
# baseline (speedup 1.0000x reference)
"""Trainium2 Bass kernel for nn_Attn (additive attention energies + softmax).

Reference computation (per batch b):
    c[g]      = sum_h Wh[g,h] * hidden[b,h] + bias[g]          (Wh = W[:, :H])
    pre[t,g]  = tanh(c[g] + sum_h enc[b,t,h] * We[g,h])        (We = W[:, H:])
    en[t]     = sum_g pre[t,g] * v[g]
    out[b,t]  = softmax_t(en)

Shapes: H=1024, B=32, T=1024.  Sharding: data-parallel over batch across 8
cores (4 batches per core); W/bias/v replicated.

Per-core kernel strategy (all matmul operands cast to fp16 on-chip; fp16
matmul streams 1 column/cycle on the PE vs fp32's 4 cycles, and fp16 keeps
the end-to-end softmax error ~1.6e-3 absmax vs ~9e-3 for bf16; build_bass
accepts mm1_dt=BF16 for a ~9% faster, less accurate variant):
  - W is DMA'd in 128-row chunks, cast to fp16 and transposed on the PE
    (128x128 identity matmuls) into WhT/WeT tiles laid out [h, g].
  - enc is DMA'd per (batch, 512-t-chunk), cast fp16, transposed on the PE
    into encT tiles [h, t].
  - MM1: psum[g=128, t=512] += WeT[h,g]^T @ encT[h,t] over 8 h-chunks.
  - ACT fuses (+c[g] bias, tanh) PSUM->SBUF in one pass (bias is
    per-partition in this layout).
  - MM2: energies[1, t=512] += v[g]^T @ tanh[g, t] with v as the stationary
    operand, accumulated over the 8 g-chunks in PSUM.
  - Softmax over t on [4, 1024] (max-sub, Exp with fused accumulated sum,
    reciprocal, scale).
"""

import numpy as np

try:
    import concourse  # noqa: F401
except ImportError:  # pragma: no cover
    import sys

    sys.path.insert(0, "/opt/trn_rl_repo")

import concourse.bass as bass  # noqa: E402
import concourse.mybir as mybir  # noqa: E402
import concourse.tile as tile  # noqa: E402
from concourse import bacc  # noqa: E402
from concourse.bass_utils import run_bass_kernel_spmd  # noqa: E402
from concourse.masks import make_identity  # noqa: E402

H = 1024
B = 32
T = 1024
N_CORES = 8
B_LOC = B // N_CORES  # 4 batches per core

F32 = mybir.dt.float32
F16 = mybir.dt.float16
BF16 = mybir.dt.bfloat16
AFT = mybir.ActivationFunctionType


def build_bass(repeat_n=None, dma_tr=False, mm1_dt=F16):
    """Build the per-core Bass program.

    repeat_n: if set, wrap the main phase in a hardware For_i loop that
    re-executes it repeat_n times.  Used only for wall-clock timing of the
    steady-state kernel body (outputs of iterations >= 2 read stale tiles, so
    the result tensor is NOT meaningful in that mode).
    """
    nc = bacc.Bacc("TRN2", target_bir_lowering=False, debug=False)

    enc = nc.dram_tensor("enc", [B_LOC, T, H], F32, kind="ExternalInput").ap()
    hid = nc.dram_tensor("hid", [B_LOC, H], F32, kind="ExternalInput").ap()
    w = nc.dram_tensor("w", [H, 2 * H], F32, kind="ExternalInput").ap()
    bias = nc.dram_tensor("bias", [H], F32, kind="ExternalInput").ap()
    v = nc.dram_tensor("v", [H], F32, kind="ExternalInput").ap()
    out = nc.dram_tensor("out", [B_LOC, T], F32, kind="ExternalOutput").ap()

    HC = H // 128  # 8 h-chunks
    GC = H // 128  # 8 g-chunks
    TCH = 512  # t-chunk (PSUM free-dim limit)
    N_ROUNDS = B_LOC * (T // TCH)  # 8 rounds of (batch, t-chunk)

    with tile.TileContext(nc) as tc:
        ctx_pools = []

        def pool(name, bufs, space="SBUF"):
            p = tc.tile_pool(name=name, bufs=bufs, space=space)
            ctx_pools.append(p)
            return p.__enter__()

        consts = pool("consts", 1)
        wt = pool("wt", 1)
        wstage = pool("wstage", 2)
        encnat = pool("encnat", 8)
        enc16p = pool("enc16", 8)
        encTp = pool("encT", 16)
        tanhp = pool("tanh", 10)
        esb = pool("esb", 1)
        dscr = pool("dscr", 2, space="DRAM")
        # PSUM: 8 banks total; 1 + 2 + 3 + 2 = 8.
        ps_c = pool("ps_c", 1, space="PSUM")
        ps_tr = pool("ps_tr", 2, space="PSUM")
        ps_mm = pool("ps_mm", 3, space="PSUM")
        ps_en = pool("ps_en", 2, space="PSUM")
        ps_w = ps_tr  # W/h transposes share the enc-transpose PSUM slots

        # ---- constants ----
        ident16 = consts.tile([128, 128], mm1_dt, tag="ident16", name="ident16")
        make_identity(nc, ident16[:])

        bias_sb = consts.tile([128, GC], F32, tag="bias_sb", name="bias_sb")
        nc.sync.dma_start(bias_sb[:], bias.rearrange("(o p) -> p o", p=128))
        vf = consts.tile([128, GC], F32, tag="vf", name="vf")
        nc.sync.dma_start(vf[:], v.rearrange("(o p) -> p o", p=128))
        v16 = consts.tile([128, GC], F16, tag="v16", name="v16")
        nc.vector.tensor_copy(v16[:], vf[:])

        # hidden -> hT (fp16), padded to 128 partitions so the PE transpose
        # uses a full-K identity matmul.
        hf = consts.tile([B_LOC, H], F32, tag="hf", name="hf")
        nc.sync.dma_start(hf[:], hid)
        h16 = consts.tile([128, H], mm1_dt, tag="h16", name="h16")
        nc.gpsimd.memset(h16[:], 0.0)
        nc.vector.tensor_copy(h16[:B_LOC, :], hf[:])
        hts = []
        for hc in range(HC):
            t = consts.tile([128, 128], mm1_dt, tag=f"hts{hc}", name=f"hts{hc}")
            if dma_tr:
                nc.scalar.dma_start(t[:], h16[:, 128 * hc : 128 * (hc + 1)], transpose=True)
            else:
                p = ps_w.tile([128, TCH], mm1_dt, tag="ps_tr", name="ps_wh")
                nc.tensor.transpose(p[:, :128], h16[:, 128 * hc : 128 * (hc + 1)], ident16[:])
                nc.vector.tensor_copy(t[:], p[:, :128])
            hts.append(t)

        # Persistent weight tiles: WhT/WeT in [h, g] layout, fp16.
        wht = [wt.tile([128, H], mm1_dt, tag=f"wht{hc}", name=f"wht{hc}") for hc in range(HC)]
        wet = [wt.tile([128, H], mm1_dt, tag=f"wet{hc}", name=f"wet{hc}") for hc in range(HC)]
        c_sb = [consts.tile([128, B_LOC], F32, tag=f"c{gi}", name=f"c{gi}") for gi in range(GC)]

        def emit_w_chunk(gi):
            """DMA W rows [128*gi, 128*(gi+1)), transpose into column gi of
            all WhT/WeT tiles, then compute c[:, :] for this g-chunk."""
            wf = wstage.tile([128, 2 * H], F32, tag="wf", name="wf")
            nc.sync.dma_start(wf[:], w[128 * gi : 128 * (gi + 1), :])
            w16 = wstage.tile([128, 2 * H], mm1_dt, tag="w16", name="w16")
            nc.vector.tensor_copy(w16[:], wf[:])
            for c in range(2 * HC):
                dst = wht[c] if c < HC else wet[c - HC]
                if dma_tr:
                    nc.scalar.dma_start(
                        dst[:, 128 * gi : 128 * (gi + 1)],
                        w16[:, 128 * c : 128 * (c + 1)],
                        transpose=True,
                    )
                else:
                    p = ps_w.tile([128, TCH], mm1_dt, tag="ps_tr", name="ps_ww")
                    nc.tensor.transpose(p[:, :128], w16[:, 128 * c : 128 * (c + 1)], ident16[:])
                    nc.vector.tensor_copy(dst[:, 128 * gi : 128 * (gi + 1)], p[:, :128])
            # c[g, b] for this g-chunk = sum_h Wh[g,h] h[b,h] + bias[g]
            pc = ps_c.tile([128, B_LOC], F32, tag="ps_c", name="ps_c")
            for hc in range(HC):
                nc.tensor.matmul(
                    pc[:],
                    wht[hc][:, 128 * gi : 128 * (gi + 1)],
                    hts[hc][:, :B_LOC],
                    start=(hc == 0),
                    stop=(hc == HC - 1),
                )
            nc.vector.tensor_scalar_add(c_sb[gi][:], pc[:], bias_sb[:, gi : gi + 1])

        def emit_prep(r):
            """DMA + cast + PE-transpose enc for round r; returns encT tiles."""
            b, tcx = divmod(r, T // TCH)
            t0 = tcx * TCH
            nat = []
            for i in range(TCH // 128):
                tl = encnat.tile([128, H], F32, tag="nat", name="nat")
                nc.sync.dma_start(tl[:], enc[b, t0 + 128 * i : t0 + 128 * (i + 1), :])
                nat.append(tl)
            e16 = []
            for i in range(TCH // 128):
                tl = enc16p.tile([128, H], mm1_dt, tag="e16", name="e16")
                nc.vector.tensor_copy(tl[:], nat[i][:])
                e16.append(tl)
            if dma_tr:
                scr = dscr.tile([TCH, H], mm1_dt, tag="scr", name="scr")
                for i in range(TCH // 128):
                    nc.sync.dma_start(scr[128 * i : 128 * (i + 1), :], e16[i][:])
            encT = []
            for hc in range(HC):
                tl = encTp.tile([128, TCH], mm1_dt, tag="encT", name="encT")
                if dma_tr:
                    nc.sync.dma_start_transpose(
                        tl[:], scr[:, 128 * hc : 128 * (hc + 1)]
                    )
                else:
                    p = ps_tr.tile([128, TCH], mm1_dt, tag="ps_tr", name="ps_tr")
                    for ti in range(TCH // 128):
                        nc.tensor.transpose(
                            p[:, 128 * ti : 128 * (ti + 1)],
                            e16[ti][:, 128 * hc : 128 * (hc + 1)],
                            ident16[:],
                        )
                    nc.vector.tensor_copy(tl[:], p[:])
                encT.append(tl)
            return encT

        # energies staging: one [1, 512] fp32 tile per round (partition 0),
        # gathered into [B_LOC, T] by SBUF->SBUF DMAs before the softmax.
        e_parts = [esb.tile([1, TCH], F32, tag=f"e{r}", name=f"e{r}") for r in range(N_ROUNDS)]
        energies = esb.tile([B_LOC, T], F32, tag="energies", name="energies")

        def emit_mm2(r, tanh_tiles):
            """v-reduction over g for round r's tanh tiles, then stage the
            energies row.  Deferred past the next round's first MM1 group so
            the PE never waits on the last tanh."""
            b, tcx = divmod(r, T // TCH)
            pen = ps_en.tile([1, TCH], F32, tag="ps_en", name="ps_en")
            for gi in range(GC):
                nc.tensor.matmul(
                    pen[:],
                    v16[:, gi : gi + 1],
                    tanh_tiles[gi][:],
                    start=(gi == 0),
                    stop=(gi == GC - 1),
                )
            nc.scalar.copy(e_parts[r][:], pen[:])
            nc.sync.dma_start(
                energies[b : b + 1, TCH * tcx : TCH * (tcx + 1)], e_parts[r][:]
            )

        def emit_main(first):
            encT_cur = emit_prep(0)
            pending_mm2 = None
            for r in range(N_ROUNDS):
                b, tcx = divmod(r, T // TCH)
                tanh_tiles = []
                for gi in range(GC):
                    if r == 0 and first:
                        emit_w_chunk(gi)
                    pm = ps_mm.tile([128, TCH], F32, tag="ps_mm", name="ps_mm")
                    for hc in range(HC):
                        nc.tensor.matmul(
                            pm[:],
                            wet[hc][:, 128 * gi : 128 * (gi + 1)],
                            encT_cur[hc][:],
                            start=(hc == 0),
                            stop=(hc == HC - 1),
                        )
                    th = tanhp.tile([128, TCH], F16, tag="tanh", name="tanh")
                    nc.scalar.activation(
                        th[:], pm[:], AFT.Tanh, bias=c_sb[gi][:, b : b + 1], scale=1.0
                    )
                    tanh_tiles.append(th)
                    if gi == 0 and pending_mm2 is not None:
                        emit_mm2(*pending_mm2)
                        pending_mm2 = None
                    if gi == 4 and r + 1 < N_ROUNDS:
                        encT_next = emit_prep(r + 1)
                pending_mm2 = (r, tanh_tiles)
                if r + 1 < N_ROUNDS:
                    encT_cur = encT_next
            emit_mm2(*pending_mm2)

            # ---- softmax over t for all 4 batches at once ----
            mx = esb.tile([B_LOC, 1], F32, tag="mx", name="mx")
            nc.vector.reduce_max(mx[:], energies[:], axis=mybir.AxisListType.X)
            nmx = esb.tile([B_LOC, 1], F32, tag="nmx", name="nmx")
            nc.vector.tensor_scalar_mul(nmx[:], mx[:], -1.0)
            ex = esb.tile([B_LOC, T], F32, tag="ex", name="ex")
            sm = esb.tile([B_LOC, 1], F32, tag="sm", name="sm")
            nc.scalar.activation(
                ex[:], energies[:], AFT.Exp, bias=nmx[:], scale=1.0, accum_out=sm[:]
            )
            rs = esb.tile([B_LOC, 1], F32, tag="rs", name="rs")
            nc.vector.reciprocal(rs[:], sm[:])
            osb = esb.tile([B_LOC, T], F32, tag="osb", name="osb")
            nc.vector.tensor_scalar_mul(osb[:], ex[:], rs[:])
            nc.sync.dma_start(out, osb[:])

        if repeat_n:
            with tc.For_i(0, repeat_n, 1):
                emit_main(first=True)
        else:
            emit_main(first=True)

        for p in reversed(ctx_pools):
            p.__exit__(None, None, None)

    nc.compile()
    return nc


_NC = None


def _get_nc():
    global _NC
    if _NC is None:
        _NC = build_bass()
    return _NC


def kernel(hidden, encoder_outputs, W, b, v):
    nc = _get_nc()
    hidden = np.asarray(hidden, dtype=np.float32)
    encoder_outputs = np.asarray(encoder_outputs, dtype=np.float32)
    W = np.asarray(W, dtype=np.float32)
    b = np.asarray(b, dtype=np.float32)
    v = np.asarray(v, dtype=np.float32)
    hid = hidden[0]  # [B, H]
    in_maps = []
    for i in range(N_CORES):
        s = slice(B_LOC * i, B_LOC * (i + 1))
        in_maps.append(
            {
                "enc": np.ascontiguousarray(encoder_outputs[s]),
                "hid": np.ascontiguousarray(hid[s]),
                "w": W,
                "bias": b,
                "v": v,
            }
        )
    res = run_bass_kernel_spmd(nc, in_maps, core_ids=list(range(N_CORES)))
    full = np.concatenate([res.results[i]["out"] for i in range(N_CORES)], axis=0)
    return full[:, None, :].astype(np.float32)

